# revision 11
# baseline (speedup 1.0000x reference)
"""Multi-head attention (12 heads, N=4096, C=768) on 8 TRN2 NeuronCores.

Sharding: 8 cores = 4 head-groups x 2 sequence halves.
  core c: heads 3*(c%4) .. 3*(c%4)+2, query rows half (c//4).
Each core computes K/V projections for its 3 heads over the FULL sequence
(inputs are passed with the core's query half rotated to the front, which is
legal because softmax+PV is permutation-invariant along the key axis), Q for
its 2048 query rows, eager attention in S^T orientation (keys on PSUM
partitions), and a partial output projection. Host sums the 4 head-group
partials per sequence half and adds the bias terms.

Bias algebra (exact): bk drops out of softmax entirely; bv contributes
bv @ Wo to every output row (added on host with bo); bq is folded into Q.

All matmuls run with bf16 inputs and fp32 PSUM accumulation.
"""

import numpy as np
import ml_dtypes

import concourse.bass as bass
from concourse import bacc
import concourse.tile as tile
import concourse.mybir as mybir
from concourse.bass_utils import run_bass_kernel_spmd

P = 128
C = 768                    # hidden
NSEQ = 4096                # sequence length
HPC = 3                    # heads per core
HD = 64                    # head dim
HW = HPC * HD              # 192, projection width per core
QB = 2048                  # query rows per core
QBLK = 1024                # query block (PSUM-friendly)
NCH = C // P               # 6 contraction chunks
KT = NSEQ // P             # 32 key tiles
BF16 = mybir.dt.bfloat16
F32 = mybir.dt.float32
AF = mybir.ActivationFunctionType
ALU = mybir.AluOpType

_CACHE = {}

# set by test.py to capture profiling info
TRACE = False
LAST_RESULT = None


def _build():
    nc = bacc.Bacc("TRN2")

    xT = nc.dram_tensor("xT", [C, NSEQ], BF16, kind="ExternalInput")
    wq = nc.dram_tensor("wq", [C, HW], BF16, kind="ExternalInput")
    wk = nc.dram_tensor("wk", [C, HW], BF16, kind="ExternalInput")
    wv = nc.dram_tensor("wv", [C, HW], BF16, kind="ExternalInput")
    wo = nc.dram_tensor("wo", [HW, C], BF16, kind="ExternalInput")
    bq = nc.dram_tensor("bq", [1, HW], BF16, kind="ExternalInput")
    out = nc.dram_tensor("out", [QB, C], F32, kind="ExternalOutput")

    with tile.TileContext(nc) as tc:
        with (
            tc.tile_pool(name="const", bufs=1) as const,
            tc.tile_pool(name="proj", bufs=1) as proj,
            tc.tile_pool(name="pt", bufs=3) as ptp,
            tc.tile_pool(name="stage", bufs=3) as stage,
            tc.tile_pool(name="psS", bufs=2, space="PSUM") as psS,
            tc.tile_pool(name="psO", bufs=1, space="PSUM") as psO,
            tc.tile_pool(name="psX", bufs=2, space="PSUM") as psX,
        ):
            # ---- load inputs ----
            xt = const.tile([P, NCH, NSEQ], BF16)
            nc.sync.dma_start(xt[:], xT[:].rearrange("(c p) n -> p c n", p=P))
            wq_sb = const.tile([P, NCH, HW], BF16)
            nc.sync.dma_start(wq_sb[:], wq[:].rearrange("(c p) m -> p c m", p=P))
            wk_sb = const.tile([P, NCH, HW], BF16)
            nc.sync.dma_start(wk_sb[:], wk[:].rearrange("(c p) m -> p c m", p=P))
            wv_sb = const.tile([P, NCH, HW], BF16)
            nc.sync.dma_start(wv_sb[:], wv[:].rearrange("(c p) m -> p c m", p=P))
            wo_sb = const.tile([HD, HPC, C], BF16)
            nc.sync.dma_start(wo_sb[:], wo[:].rearrange("(h d) n -> d h n", d=HD))
            bq_sb = const.tile([1, HW], BF16)
            nc.sync.dma_start(bq_sb[:], bq[:])
            ones_row = const.tile([1, 512], BF16)
            nc.vector.memset(ones_row[:], 1.0)
            ones_f32 = const.tile([P, HD], F32)
            nc.vector.memset(ones_f32[:], 1.0)

            # ---- persistent projection outputs ----
            KT01 = proj.tile([P, NSEQ], BF16)      # heads 0,1 K^T stacked
            KT2 = proj.tile([HD, NSEQ], BF16)      # head 2 K^T
            QT01 = proj.tile([P, QB], BF16)
            QT2 = proj.tile([HD, QB], BF16)
            V_sb = proj.tile([P, KT, HPC, HD + 1], BF16)  # V + ones column
            attnT = proj.tile([HD, HPC, 2, QBLK], BF16)   # normalized attn^T

            nc.vector.memset(V_sb[:, :, :, HD : HD + 1], 1.0)

            # ---- K projection: K^T[d, n] = sum_c Wk[c, d] * xT[c, n] ----
            for nt in range(NSEQ // 512):
                ps = psX.tile([P, 512], F32, tag="ps")
                for c in range(NCH):
                    nc.tensor.matmul(
                        ps[:], wk_sb[:, c, 0:P], xt[:, c, nt * 512 : (nt + 1) * 512],
                        start=(c == 0), stop=(c == NCH - 1),
                    )
                nc.vector.tensor_copy(KT01[:, nt * 512 : (nt + 1) * 512], ps[:])
                ps2 = psX.tile([P, 512], F32, tag="ps")
                for c in range(NCH):
                    nc.tensor.matmul(
                        ps2[0:HD, :], wk_sb[:, c, P:HW], xt[:, c, nt * 512 : (nt + 1) * 512],
                        start=(c == 0), stop=(c == NCH - 1),
                    )
                nc.vector.tensor_copy(KT2[:, nt * 512 : (nt + 1) * 512], ps2[0:HD, :])

            # ---- Q projection (first QB columns of xT); bias folded in as a
            # rank-1 accumulation: Q^T += bq^T @ ones ----
            for nt in range(QB // 512):
                ps = psX.tile([P, 512], F32, tag="ps")
                for c in range(NCH):
                    nc.tensor.matmul(
                        ps[:], wq_sb[:, c, 0:P], xt[:, c, nt * 512 : (nt + 1) * 512],
                        start=(c == 0), stop=False,
                    )
                nc.tensor.matmul(
                    ps[:], bq_sb[0:1, 0:P], ones_row[0:1, :], start=False, stop=True
                )
                nc.vector.tensor_copy(QT01[:, nt * 512 : (nt + 1) * 512], ps[:])
                ps2 = psX.tile([P, 512], F32, tag="ps")
                for c in range(NCH):
                    nc.tensor.matmul(
                        ps2[0:HD, :], wq_sb[:, c, P:HW], xt[:, c, nt * 512 : (nt + 1) * 512],
                        start=(c == 0), stop=False,
                    )
                nc.tensor.matmul(
                    ps2[0:HD, :], bq_sb[0:1, P:HW], ones_row[0:1, :], start=False, stop=True
                )
                nc.vector.tensor_copy(QT2[:, nt * 512 : (nt + 1) * 512], ps2[0:HD, :])

            # ---- V projection: V[n, m] = sum_c xT[c, n] * Wv[c, m] ----
            for kt in range(KT):
                ps = psX.tile([P, 512], F32, tag="ps")
                for c in range(NCH):
                    nc.tensor.matmul(
                        ps[:, 0:HW], xt[:, c, kt * P : (kt + 1) * P], wv_sb[:, c, :],
                        start=(c == 0), stop=(c == NCH - 1),
                    )
                nc.vector.tensor_copy(
                    V_sb[:, kt, :, 0:HD],
                    ps[:, 0:HW].rearrange("p (h d) -> p h d", d=HD),
                )

            # ---- attention units: (head, qblock) ----
            def kt_ap(h, kt):
                if h == 0:
                    return KT01[0:HD, kt * P : (kt + 1) * P]
                if h == 1:
                    return KT01[HD:P, kt * P : (kt + 1) * P]
                return KT2[:, kt * P : (kt + 1) * P]

            def qt_ap(h, qb, qt):
                lo = qb * QBLK + qt * 512
                if h == 0:
                    return QT01[0:HD, lo : lo + 512]
                if h == 1:
                    return QT01[HD:P, lo : lo + 512]
                return QT2[:, lo : lo + 512]

            for qb in range(2):
                for h in range(HPC):
                    O_t = psO.tile([HD + 1, QBLK], F32, tag="o")
                    for kt in range(KT):
                        S_t = psS.tile([P, QBLK], F32, tag="s")
                        for qt in range(2):
                            nc.tensor.matmul(
                                S_t[:, qt * 512 : (qt + 1) * 512],
                                kt_ap(h, kt), qt_ap(h, qb, qt),
                                start=True, stop=True,
                            )
                        pt = ptp.tile([P, QBLK], BF16, tag="pt")
                        nc.scalar.activation(pt[:], S_t[:], AF.Exp, scale=0.125)
                        for qt in range(2):
                            nc.tensor.matmul(
                                O_t[:, qt * 512 : (qt + 1) * 512],
                                V_sb[:, kt, h, :], pt[:, qt * 512 : (qt + 1) * 512],
                                start=(kt == 0), stop=(kt == KT - 1),
                            )
                    # normalize: row HD of O_t is the softmax denominator.
                    # Broadcast 1/den across partitions with a K=1 PE matmul
                    # (ones[1,HD]^T @ recip[1,QBLK]), then multiply.
                    den = stage.tile([P, QBLK], F32, tag="den")
                    nc.vector.reciprocal(den[HD : HD + 1, :], O_t[HD : HD + 1, :])
                    den_ps = psS.tile([P, QBLK], F32, tag="s")
                    for qt in range(2):
                        nc.tensor.matmul(
                            den_ps[0:HD, qt * 512 : (qt + 1) * 512],
                            ones_f32[HD : HD + 1, 0:HD],
                            den[HD : HD + 1, qt * 512 : (qt + 1) * 512],
                            start=True, stop=True,
                        )
                    O_sb = stage.tile([P, QBLK], F32, tag="osb")
                    nc.vector.tensor_copy(O_sb[0:HD, :], O_t[0:HD, :])
                    nc.vector.tensor_tensor(
                        attnT[:, h, qb, :], O_sb[0:HD, :], den_ps[0:HD, :], ALU.mult
                    )

            # ---- output projection: out[q, n] = sum_h attnT[d, h, q] @ wo[d, h, n] ----
            for qb in range(2):
                for qs in range(QBLK // P):
                    for s0, sw in ((0, 512), (512, 256)):
                        psw = psX.tile([P, 512], F32, tag="ps")
                        for h in range(HPC):
                            nc.tensor.matmul(
                                psw[:, 0:sw],
                                attnT[:, h, qb, qs * P : (qs + 1) * P],
                                wo_sb[:, h, s0 : s0 + sw],
                                start=(h == 0), stop=(h == HPC - 1),
                            )
                        st = stage.tile([P, 512], F32, tag="st")
                        nc.vector.tensor_copy(st[:, 0:sw], psw[:, 0:sw])
                        nc.sync.dma_start(
                            out[qb * QBLK + qs * P : qb * QBLK + (qs + 1) * P, s0 : s0 + sw],
                            st[:, 0:sw],
                        )

    if hasattr(nc, "compile"):
        nc.compile()
    return nc


def _get_nc():
    if "nc" not in _CACHE:
        _CACHE["nc"] = _build()
    return _CACHE["nc"]


def kernel(x, Wq, bq, Wk, bk, Wv, bv, Wo, bo):
    global LAST_RESULT
    x = np.asarray(x, dtype=np.float32)
    Wq = np.asarray(Wq, dtype=np.float32)
    Wk = np.asarray(Wk, dtype=np.float32)
    Wv = np.asarray(Wv, dtype=np.float32)
    Wo = np.asarray(Wo, dtype=np.float32)
    bq = np.asarray(bq, dtype=np.float32)
    bv = np.asarray(bv, dtype=np.float32)
    bo = np.asarray(bo, dtype=np.float32)

    B, N, Ch = x.shape
    assert (B, N, Ch) == (1, NSEQ, C)
    xT_full = np.ascontiguousarray(x[0].T)  # [C, N] f32

    bf = ml_dtypes.bfloat16
    in_maps = []
    for c in range(8):
        qhalf = c // 4
        hbase = HPC * (c % 4)
        cols = slice(hbase * HD, hbase * HD + HW)
        if qhalf == 0:
            xTc = xT_full
        else:
            xTc = np.concatenate([xT_full[:, QB:], xT_full[:, :QB]], axis=1)
        bq_in = np.ascontiguousarray(bq[cols].reshape(1, HW)).astype(bf)
        in_maps.append({
            "xT": np.ascontiguousarray(xTc).astype(bf),
            "wq": np.ascontiguousarray(Wq[:, cols]).astype(bf),
            "wk": np.ascontiguousarray(Wk[:, cols]).astype(bf),
            "wv": np.ascontiguousarray(Wv[:, cols]).astype(bf),
            "wo": np.ascontiguousarray(Wo[cols, :]).astype(bf),
            "bq": bq_in,
        })

    nc = _get_nc()
    res = run_bass_kernel_spmd(nc, in_maps, core_ids=list(range(8)), trace=TRACE)
    LAST_RESULT = res

    out = np.zeros((NSEQ, C), np.float32)
    for c in range(4):
        out[:QB] += res.results[c]["out"]
    for c in range(4, 8):
        out[QB:] += res.results[c]["out"]
    out += bo + bv @ Wo
    return out.reshape(1, NSEQ, C)


# revision 12
# speedup vs baseline: 11778.2185x; 11778.2185x over previous
"""Multi-head attention (12 heads, N=4096, C=768) on 8 TRN2 NeuronCores.

Sharding: 8 cores = 4 head-groups x 2 sequence halves.
  core c: heads 3*(c%4) .. 3*(c%4)+2, query rows half (c//4).
Each core computes K/V projections for its 3 heads over the FULL sequence
(inputs are passed with the core's query half rotated to the front, which is
legal because softmax+PV is permutation-invariant along the key axis), Q for
its 2048 query rows, eager attention in S^T orientation (keys on PSUM
partitions), and a partial output projection. Host sums the 4 head-group
partials per sequence half and adds the bias terms.

Bias algebra (exact): bk drops out of softmax entirely; bv contributes
bv @ Wo to every output row (added on host with bo); bq is folded into Q.

All matmuls run with bf16 inputs and fp32 PSUM accumulation.
"""

import numpy as np
import ml_dtypes

import concourse.bass as bass
from concourse import bacc
import concourse.tile as tile
import concourse.mybir as mybir
from concourse.bass_utils import run_bass_kernel_spmd

P = 128
C = 768                    # hidden
NSEQ = 4096                # sequence length
HPC = 3                    # heads per core
HD = 64                    # head dim
HW = HPC * HD              # 192, projection width per core
QB = 2048                  # query rows per core
QBLK = 1024                # query block (PSUM-friendly)
NCH = C // P               # 6 contraction chunks
KT = NSEQ // P             # 32 key tiles
BF16 = mybir.dt.bfloat16
F32 = mybir.dt.float32
AF = mybir.ActivationFunctionType
ALU = mybir.AluOpType

_CACHE = {}

# set by test.py to capture profiling info
TRACE = False
LAST_RESULT = None


def _build():
    nc = bacc.Bacc("TRN2")

    xT = nc.dram_tensor("xT", [C, NSEQ], BF16, kind="ExternalInput")
    wq = nc.dram_tensor("wq", [C, HW], BF16, kind="ExternalInput")
    wk = nc.dram_tensor("wk", [C, HW], BF16, kind="ExternalInput")
    wv = nc.dram_tensor("wv", [C, HW], BF16, kind="ExternalInput")
    wo = nc.dram_tensor("wo", [HW, C], BF16, kind="ExternalInput")
    bq = nc.dram_tensor("bq", [1, HW], BF16, kind="ExternalInput")
    out = nc.dram_tensor("out", [QB, C], F32, kind="ExternalOutput")

    with tile.TileContext(nc) as tc:
        with (
            tc.tile_pool(name="const", bufs=1) as const,
            tc.tile_pool(name="proj", bufs=1) as proj,
            tc.tile_pool(name="pt", bufs=3) as ptp,
            tc.tile_pool(name="stage", bufs=3) as stage,
            tc.tile_pool(name="psS", bufs=2, space="PSUM") as psS,
            tc.tile_pool(name="psO", bufs=1, space="PSUM") as psO,
            tc.tile_pool(name="psX", bufs=2, space="PSUM") as psX,
        ):
            # ---- load inputs ----
            xt = const.tile([P, NCH, NSEQ], BF16)
            nc.sync.dma_start(xt[:], xT[:].rearrange("(c p) n -> p c n", p=P))
            wq_sb = const.tile([P, NCH, HW], BF16)
            nc.sync.dma_start(wq_sb[:], wq[:].rearrange("(c p) m -> p c m", p=P))
            wk_sb = const.tile([P, NCH, HW], BF16)
            nc.sync.dma_start(wk_sb[:], wk[:].rearrange("(c p) m -> p c m", p=P))
            wv_sb = const.tile([P, NCH, HW], BF16)
            nc.sync.dma_start(wv_sb[:], wv[:].rearrange("(c p) m -> p c m", p=P))
            wo_sb = const.tile([HD, HPC, C], BF16)
            nc.sync.dma_start(wo_sb[:], wo[:].rearrange("(h d) n -> d h n", d=HD))
            bq_sb = const.tile([1, HW], BF16)
            nc.sync.dma_start(bq_sb[:], bq[:])
            ones_row = const.tile([1, 512], BF16)
            nc.vector.memset(ones_row[:], 1.0)
            ones_f32 = const.tile([P, HD], F32)
            nc.vector.memset(ones_f32[:], 1.0)

            # ---- persistent projection outputs ----
            KT01 = proj.tile([P, NSEQ], BF16)      # heads 0,1 K^T stacked
            KT2 = proj.tile([HD, NSEQ], BF16)      # head 2 K^T
            QT01 = proj.tile([P, QB], BF16)
            QT2 = proj.tile([HD, QB], BF16)
            V_sb = proj.tile([P, KT, HPC, HD + 1], BF16)  # V + ones column
            attnT = proj.tile([HD, HPC, 2, QBLK], BF16)   # normalized attn^T
            O_all = proj.tile([HD + 1, HPC, 2, QBLK], F32)  # unnormalized PV + den

            nc.vector.memset(V_sb[:, :, :, HD : HD + 1], 1.0)

            # ---- K projection: K^T[d, n] = sum_c Wk[c, d] * xT[c, n] ----
            for nt in range(NSEQ // 512):
                ps = psX.tile([P, 512], F32, tag="ps")
                for c in range(NCH):
                    nc.tensor.matmul(
                        ps[:], wk_sb[:, c, 0:P], xt[:, c, nt * 512 : (nt + 1) * 512],
                        start=(c == 0), stop=(c == NCH - 1),
                    )
                nc.vector.tensor_copy(KT01[:, nt * 512 : (nt + 1) * 512], ps[:])
                ps2 = psX.tile([P, 512], F32, tag="ps")
                for c in range(NCH):
                    nc.tensor.matmul(
                        ps2[0:HD, :], wk_sb[:, c, P:HW], xt[:, c, nt * 512 : (nt + 1) * 512],
                        start=(c == 0), stop=(c == NCH - 1),
                    )
                nc.vector.tensor_copy(KT2[:, nt * 512 : (nt + 1) * 512], ps2[0:HD, :])

            # ---- Q projection (first QB columns of xT); bias folded in as a
            # rank-1 accumulation: Q^T += bq^T @ ones ----
            for nt in range(QB // 512):
                ps = psX.tile([P, 512], F32, tag="ps")
                for c in range(NCH):
                    nc.tensor.matmul(
                        ps[:], wq_sb[:, c, 0:P], xt[:, c, nt * 512 : (nt + 1) * 512],
                        start=(c == 0), stop=False,
                    )
                nc.tensor.matmul(
                    ps[:], bq_sb[0:1, 0:P], ones_row[0:1, :], start=False, stop=True
                )
                nc.vector.tensor_copy(QT01[:, nt * 512 : (nt + 1) * 512], ps[:])
                ps2 = psX.tile([P, 512], F32, tag="ps")
                for c in range(NCH):
                    nc.tensor.matmul(
                        ps2[0:HD, :], wq_sb[:, c, P:HW], xt[:, c, nt * 512 : (nt + 1) * 512],
                        start=(c == 0), stop=False,
                    )
                nc.tensor.matmul(
                    ps2[0:HD, :], bq_sb[0:1, P:HW], ones_row[0:1, :], start=False, stop=True
                )
                nc.vector.tensor_copy(QT2[:, nt * 512 : (nt + 1) * 512], ps2[0:HD, :])

            # ---- V projection: V[n, m] = sum_c xT[c, n] * Wv[c, m] ----
            for kt in range(KT):
                ps = psX.tile([P, 512], F32, tag="ps")
                for c in range(NCH):
                    nc.tensor.matmul(
                        ps[:, 0:HW], xt[:, c, kt * P : (kt + 1) * P], wv_sb[:, c, :],
                        start=(c == 0), stop=(c == NCH - 1),
                    )
                nc.vector.tensor_copy(
                    V_sb[:, kt, :, 0:HD],
                    ps[:, 0:HW].rearrange("p (h d) -> p h d", d=HD),
                )

            # ---- attention units: (head, qblock) ----
            def kt_ap(h, kt):
                if h == 0:
                    return KT01[0:HD, kt * P : (kt + 1) * P]
                if h == 1:
                    return KT01[HD:P, kt * P : (kt + 1) * P]
                return KT2[:, kt * P : (kt + 1) * P]

            def qt_ap(h, qb, qt):
                lo = qb * QBLK + qt * 512
                if h == 0:
                    return QT01[0:HD, lo : lo + 512]
                if h == 1:
                    return QT01[HD:P, lo : lo + 512]
                return QT2[:, lo : lo + 512]

            for qb in range(2):
                for h in range(HPC):
                    O_t = psO.tile([HD + 1, QBLK], F32, tag="o")
                    for kt in range(KT):
                        S_t = psS.tile([P, QBLK], F32, tag="s")
                        for qt in range(2):
                            nc.tensor.matmul(
                                S_t[:, qt * 512 : (qt + 1) * 512],
                                kt_ap(h, kt), qt_ap(h, qb, qt),
                                start=True, stop=True,
                            )
                        pt = ptp.tile([P, QBLK], BF16, tag="pt")
                        nc.scalar.activation(pt[:], S_t[:], AF.Exp, scale=0.125)
                        for qt in range(2):
                            nc.tensor.matmul(
                                O_t[:, qt * 512 : (qt + 1) * 512],
                                V_sb[:, kt, h, :], pt[:, qt * 512 : (qt + 1) * 512],
                                start=(kt == 0), stop=(kt == KT - 1),
                            )
                    # evacuate the PSUM accumulator quickly; normalization
                    # happens later, off the attention critical path
                    nc.vector.tensor_copy(O_all[:, h, qb, :], O_t[:])

            # ---- deferred softmax normalization (overlaps attention above in
            # the schedule): row HD holds the denominator; broadcast 1/den
            # across partitions with a K=1 PE matmul, then multiply ----
            for qb in range(2):
                for h in range(HPC):
                    den = stage.tile([P, QBLK], F32, tag="den")
                    nc.vector.reciprocal(den[HD : HD + 1, :], O_all[HD : HD + 1, h, qb, :])
                    den_ps = psS.tile([P, QBLK], F32, tag="s")
                    for qt in range(2):
                        nc.tensor.matmul(
                            den_ps[0:HD, qt * 512 : (qt + 1) * 512],
                            ones_f32[HD : HD + 1, 0:HD],
                            den[HD : HD + 1, qt * 512 : (qt + 1) * 512],
                            start=True, stop=True,
                        )
                    nc.vector.tensor_tensor(
                        attnT[:, h, qb, :], O_all[0:HD, h, qb, :], den_ps[0:HD, :], ALU.mult
                    )

            # ---- output projection: out[q, n] = sum_h attnT[d, h, q] @ wo[d, h, n] ----
            for qb in range(2):
                for qs in range(QBLK // P):
                    for s0, sw in ((0, 512), (512, 256)):
                        psw = psX.tile([P, 512], F32, tag="ps")
                        for h in range(HPC):
                            nc.tensor.matmul(
                                psw[:, 0:sw],
                                attnT[:, h, qb, qs * P : (qs + 1) * P],
                                wo_sb[:, h, s0 : s0 + sw],
                                start=(h == 0), stop=(h == HPC - 1),
                            )
                        st = stage.tile([P, 512], F32, tag="st")
                        nc.vector.tensor_copy(st[:, 0:sw], psw[:, 0:sw])
                        nc.sync.dma_start(
                            out[qb * QBLK + qs * P : qb * QBLK + (qs + 1) * P, s0 : s0 + sw],
                            st[:, 0:sw],
                        )

    if hasattr(nc, "compile"):
        nc.compile()
    return nc


def _get_nc():
    if "nc" not in _CACHE:
        _CACHE["nc"] = _build()
    return _CACHE["nc"]


def kernel(x, Wq, bq, Wk, bk, Wv, bv, Wo, bo):
    global LAST_RESULT
    x = np.asarray(x, dtype=np.float32)
    Wq = np.asarray(Wq, dtype=np.float32)
    Wk = np.asarray(Wk, dtype=np.float32)
    Wv = np.asarray(Wv, dtype=np.float32)
    Wo = np.asarray(Wo, dtype=np.float32)
    bq = np.asarray(bq, dtype=np.float32)
    bv = np.asarray(bv, dtype=np.float32)
    bo = np.asarray(bo, dtype=np.float32)

    B, N, Ch = x.shape
    assert (B, N, Ch) == (1, NSEQ, C)
    xT_full = np.ascontiguousarray(x[0].T)  # [C, N] f32

    bf = ml_dtypes.bfloat16
    in_maps = []
    for c in range(8):
        qhalf = c // 4
        hbase = HPC * (c % 4)
        cols = slice(hbase * HD, hbase * HD + HW)
        if qhalf == 0:
            xTc = xT_full
        else:
            xTc = np.concatenate([xT_full[:, QB:], xT_full[:, :QB]], axis=1)
        bq_in = np.ascontiguousarray(bq[cols].reshape(1, HW)).astype(bf)
        in_maps.append({
            "xT": np.ascontiguousarray(xTc).astype(bf),
            "wq": np.ascontiguousarray(Wq[:, cols]).astype(bf),
            "wk": np.ascontiguousarray(Wk[:, cols]).astype(bf),
            "wv": np.ascontiguousarray(Wv[:, cols]).astype(bf),
            "wo": np.ascontiguousarray(Wo[cols, :]).astype(bf),
            "bq": bq_in,
        })

    nc = _get_nc()
    res = run_bass_kernel_spmd(nc, in_maps, core_ids=list(range(8)), trace=TRACE)
    LAST_RESULT = res

    out = np.zeros((NSEQ, C), np.float32)
    for c in range(4):
        out[:QB] += res.results[c]["out"]
    for c in range(4, 8):
        out[QB:] += res.results[c]["out"]
    out += bo + bv @ Wo
    return out.reshape(1, NSEQ, C)


# revision 13
# speedup vs baseline: 11987.0000x; 1.0177x over previous
"""Multi-head attention (12 heads, N=4096, C=768) on 8 TRN2 NeuronCores.

Sharding: 8 cores = 4 head-groups x 2 sequence halves.
  core c: heads 3*(c%4) .. 3*(c%4)+2, query rows half (c//4).
Each core computes K/V projections for its 3 heads over the FULL sequence
(inputs are passed with the core's query half rotated to the front, which is
legal because softmax+PV is permutation-invariant along the key axis), Q for
its 2048 query rows, eager attention in S^T orientation (keys on PSUM
partitions), and a partial output projection. Host sums the 4 head-group
partials per sequence half and adds the bias terms.

Bias algebra (exact): bk drops out of softmax entirely; bv contributes
bv @ Wo to every output row (added on host with bo); bq is folded into Q.

All matmuls run with bf16 inputs and fp32 PSUM accumulation.
"""

import numpy as np
import ml_dtypes

import concourse.bass as bass
from concourse import bacc
import concourse.tile as tile
import concourse.mybir as mybir
from concourse.bass_utils import run_bass_kernel_spmd

P = 128
C = 768                    # hidden
NSEQ = 4096                # sequence length
HPC = 3                    # heads per core
HD = 64                    # head dim
HW = HPC * HD              # 192, projection width per core
QB = 2048                  # query rows per core
QBLK = 1024                # query block (PSUM-friendly)
NCH = C // P               # 6 contraction chunks
KT = NSEQ // P             # 32 key tiles
BF16 = mybir.dt.bfloat16
F32 = mybir.dt.float32
AF = mybir.ActivationFunctionType
ALU = mybir.AluOpType

_CACHE = {}

# set by test.py to capture profiling info
TRACE = False
LAST_RESULT = None


def _build():
    nc = bacc.Bacc("TRN2")

    xT = nc.dram_tensor("xT", [C, NSEQ], BF16, kind="ExternalInput")
    wq = nc.dram_tensor("wq", [C, HW], BF16, kind="ExternalInput")
    wk = nc.dram_tensor("wk", [C, HW], BF16, kind="ExternalInput")
    wv = nc.dram_tensor("wv", [C, HW], BF16, kind="ExternalInput")
    wo = nc.dram_tensor("wo", [HW, C], BF16, kind="ExternalInput")
    bq = nc.dram_tensor("bq", [1, HW], BF16, kind="ExternalInput")
    out = nc.dram_tensor("out", [QB, C], F32, kind="ExternalOutput")

    with tile.TileContext(nc) as tc:
        with (
            tc.tile_pool(name="const", bufs=1) as const,
            tc.tile_pool(name="proj", bufs=1) as proj,
            tc.tile_pool(name="pt", bufs=6) as ptp,
            tc.tile_pool(name="stage", bufs=4) as stage,
            tc.tile_pool(name="psS", bufs=2, space="PSUM") as psS,
            tc.tile_pool(name="psO", bufs=1, space="PSUM") as psO,
            tc.tile_pool(name="psX", bufs=2, space="PSUM") as psX,
        ):
            # ---- load inputs ----
            xt = const.tile([P, NCH, NSEQ], BF16)
            nc.sync.dma_start(xt[:], xT[:].rearrange("(c p) n -> p c n", p=P))
            wq_sb = const.tile([P, NCH, HW], BF16)
            nc.sync.dma_start(wq_sb[:], wq[:].rearrange("(c p) m -> p c m", p=P))
            wk_sb = const.tile([P, NCH, HW], BF16)
            nc.sync.dma_start(wk_sb[:], wk[:].rearrange("(c p) m -> p c m", p=P))
            wv_sb = const.tile([P, NCH, HW], BF16)
            nc.sync.dma_start(wv_sb[:], wv[:].rearrange("(c p) m -> p c m", p=P))
            wo_sb = const.tile([HD, HPC, C], BF16)
            nc.sync.dma_start(wo_sb[:], wo[:].rearrange("(h d) n -> d h n", d=HD))
            bq_sb = const.tile([1, HW], BF16)
            nc.sync.dma_start(bq_sb[:], bq[:])
            ones_row = const.tile([1, 512], BF16)
            nc.vector.memset(ones_row[:], 1.0)
            ones_f32 = const.tile([P, HD], F32)
            nc.vector.memset(ones_f32[:], 1.0)

            # ---- persistent projection outputs ----
            KT01 = proj.tile([P, NSEQ], BF16)      # heads 0,1 K^T stacked
            KT2 = proj.tile([HD, NSEQ], BF16)      # head 2 K^T
            QT01 = proj.tile([P, QB], BF16)
            QT2 = proj.tile([HD, QB], BF16)
            V_sb = proj.tile([P, KT, HPC, HD + 1], BF16)  # V + ones column
            attnT = proj.tile([HD, HPC, 2, QBLK], BF16)   # normalized attn^T
            O_all = proj.tile([HD + 1, HPC, 2, QBLK], F32)  # unnormalized PV + den

            nc.vector.memset(V_sb[:, :, :, HD : HD + 1], 1.0)

            # ---- K projection: K^T[d, n] = sum_c Wk[c, d] * xT[c, n] ----
            for nt in range(NSEQ // 512):
                ps = psX.tile([P, 512], F32, tag="ps")
                for c in range(NCH):
                    nc.tensor.matmul(
                        ps[:], wk_sb[:, c, 0:P], xt[:, c, nt * 512 : (nt + 1) * 512],
                        start=(c == 0), stop=(c == NCH - 1),
                    )
                nc.vector.tensor_copy(KT01[:, nt * 512 : (nt + 1) * 512], ps[:])
                ps2 = psX.tile([P, 512], F32, tag="ps")
                for c in range(NCH):
                    nc.tensor.matmul(
                        ps2[0:HD, :], wk_sb[:, c, P:HW], xt[:, c, nt * 512 : (nt + 1) * 512],
                        start=(c == 0), stop=(c == NCH - 1),
                    )
                nc.vector.tensor_copy(KT2[:, nt * 512 : (nt + 1) * 512], ps2[0:HD, :])

            # ---- Q projection (first QB columns of xT); bias folded in as a
            # rank-1 accumulation: Q^T += bq^T @ ones ----
            for nt in range(QB // 512):
                ps = psX.tile([P, 512], F32, tag="ps")
                for c in range(NCH):
                    nc.tensor.matmul(
                        ps[:], wq_sb[:, c, 0:P], xt[:, c, nt * 512 : (nt + 1) * 512],
                        start=(c == 0), stop=False,
                    )
                nc.tensor.matmul(
                    ps[:], bq_sb[0:1, 0:P], ones_row[0:1, :], start=False, stop=True
                )
                nc.vector.tensor_copy(QT01[:, nt * 512 : (nt + 1) * 512], ps[:])
                ps2 = psX.tile([P, 512], F32, tag="ps")
                for c in range(NCH):
                    nc.tensor.matmul(
                        ps2[0:HD, :], wq_sb[:, c, P:HW], xt[:, c, nt * 512 : (nt + 1) * 512],
                        start=(c == 0), stop=False,
                    )
                nc.tensor.matmul(
                    ps2[0:HD, :], bq_sb[0:1, P:HW], ones_row[0:1, :], start=False, stop=True
                )
                nc.vector.tensor_copy(QT2[:, nt * 512 : (nt + 1) * 512], ps2[0:HD, :])

            # ---- V projection: V[n, m] = sum_c xT[c, n] * Wv[c, m] ----
            for kt in range(KT):
                ps = psX.tile([P, 512], F32, tag="ps")
                for c in range(NCH):
                    nc.tensor.matmul(
                        ps[:, 0:HW], xt[:, c, kt * P : (kt + 1) * P], wv_sb[:, c, :],
                        start=(c == 0), stop=(c == NCH - 1),
                    )
                nc.vector.tensor_copy(
                    V_sb[:, kt, :, 0:HD],
                    ps[:, 0:HW].rearrange("p (h d) -> p h d", d=HD),
                )

            # ---- attention units: (head, qblock) ----
            def kt_ap(h, kt):
                if h == 0:
                    return KT01[0:HD, kt * P : (kt + 1) * P]
                if h == 1:
                    return KT01[HD:P, kt * P : (kt + 1) * P]
                return KT2[:, kt * P : (kt + 1) * P]

            def qt_ap(h, qb, qt):
                lo = qb * QBLK + qt * 512
                if h == 0:
                    return QT01[0:HD, lo : lo + 512]
                if h == 1:
                    return QT01[HD:P, lo : lo + 512]
                return QT2[:, lo : lo + 512]

            for qb in range(2):
                for h in range(HPC):
                    O_t = psO.tile([HD + 1, QBLK], F32, tag="o")
                    for kt in range(KT):
                        S_t = psS.tile([P, QBLK], F32, tag="s")
                        for qt in range(2):
                            nc.tensor.matmul(
                                S_t[:, qt * 512 : (qt + 1) * 512],
                                kt_ap(h, kt), qt_ap(h, qb, qt),
                                start=True, stop=True,
                            )
                        pt = ptp.tile([P, QBLK], BF16, tag="pt")
                        nc.scalar.activation(pt[:], S_t[:], AF.Exp, scale=0.125)
                        for qt in range(2):
                            nc.tensor.matmul(
                                O_t[:, qt * 512 : (qt + 1) * 512],
                                V_sb[:, kt, h, :], pt[:, qt * 512 : (qt + 1) * 512],
                                start=(kt == 0), stop=(kt == KT - 1),
                            )
                    # evacuate the PSUM accumulator quickly; normalization
                    # happens later, off the attention critical path
                    nc.vector.tensor_copy(O_all[:, h, qb, :], O_t[:])

            # ---- deferred softmax normalization (overlaps attention above in
            # the schedule): row HD holds the denominator; broadcast 1/den
            # across partitions with a K=1 PE matmul, then multiply ----
            for qb in range(2):
                for h in range(HPC):
                    den = stage.tile([P, QBLK], F32, tag="den")
                    nc.vector.reciprocal(den[HD : HD + 1, :], O_all[HD : HD + 1, h, qb, :])
                    den_ps = psS.tile([P, QBLK], F32, tag="s")
                    for qt in range(2):
                        nc.tensor.matmul(
                            den_ps[0:HD, qt * 512 : (qt + 1) * 512],
                            ones_f32[HD : HD + 1, 0:HD],
                            den[HD : HD + 1, qt * 512 : (qt + 1) * 512],
                            start=True, stop=True,
                        )
                    nc.vector.tensor_tensor(
                        attnT[:, h, qb, :], O_all[0:HD, h, qb, :], den_ps[0:HD, :], ALU.mult
                    )

            # ---- output projection: out[q, n] = sum_h attnT[d, h, q] @ wo[d, h, n] ----
            for qb in range(2):
                for qs in range(QBLK // P):
                    for s0, sw in ((0, 512), (512, 256)):
                        psw = psX.tile([P, 512], F32, tag="ps")
                        for h in range(HPC):
                            nc.tensor.matmul(
                                psw[:, 0:sw],
                                attnT[:, h, qb, qs * P : (qs + 1) * P],
                                wo_sb[:, h, s0 : s0 + sw],
                                start=(h == 0), stop=(h == HPC - 1),
                            )
                        st = stage.tile([P, 512], F32, tag="st")
                        nc.vector.tensor_copy(st[:, 0:sw], psw[:, 0:sw])
                        nc.sync.dma_start(
                            out[qb * QBLK + qs * P : qb * QBLK + (qs + 1) * P, s0 : s0 + sw],
                            st[:, 0:sw],
                        )

    if hasattr(nc, "compile"):
        nc.compile()
    return nc


def _get_nc():
    if "nc" not in _CACHE:
        _CACHE["nc"] = _build()
    return _CACHE["nc"]


def kernel(x, Wq, bq, Wk, bk, Wv, bv, Wo, bo):
    global LAST_RESULT
    x = np.asarray(x, dtype=np.float32)
    Wq = np.asarray(Wq, dtype=np.float32)
    Wk = np.asarray(Wk, dtype=np.float32)
    Wv = np.asarray(Wv, dtype=np.float32)
    Wo = np.asarray(Wo, dtype=np.float32)
    bq = np.asarray(bq, dtype=np.float32)
    bv = np.asarray(bv, dtype=np.float32)
    bo = np.asarray(bo, dtype=np.float32)

    B, N, Ch = x.shape
    assert (B, N, Ch) == (1, NSEQ, C)
    xT_full = np.ascontiguousarray(x[0].T)  # [C, N] f32

    bf = ml_dtypes.bfloat16
    in_maps = []
    for c in range(8):
        qhalf = c // 4
        hbase = HPC * (c % 4)
        cols = slice(hbase * HD, hbase * HD + HW)
        if qhalf == 0:
            xTc = xT_full
        else:
            xTc = np.concatenate([xT_full[:, QB:], xT_full[:, :QB]], axis=1)
        bq_in = np.ascontiguousarray(bq[cols].reshape(1, HW)).astype(bf)
        in_maps.append({
            "xT": np.ascontiguousarray(xTc).astype(bf),
            "wq": np.ascontiguousarray(Wq[:, cols]).astype(bf),
            "wk": np.ascontiguousarray(Wk[:, cols]).astype(bf),
            "wv": np.ascontiguousarray(Wv[:, cols]).astype(bf),
            "wo": np.ascontiguousarray(Wo[cols, :]).astype(bf),
            "bq": bq_in,
        })

    nc = _get_nc()
    res = run_bass_kernel_spmd(nc, in_maps, core_ids=list(range(8)), trace=TRACE)
    LAST_RESULT = res

    out = np.zeros((NSEQ, C), np.float32)
    for c in range(4):
        out[:QB] += res.results[c]["out"]
    for c in range(4, 8):
        out[QB:] += res.results[c]["out"]
    out += bo + bv @ Wo
    return out.reshape(1, NSEQ, C)
